# revision 1
# baseline (speedup 1.0000x reference)
"""Trainium2 Bass kernel for nn_Attention_17334488007435.

Cosine-ish spatial attention:
  qkv = w_qkv @ x   (1x1 conv)
  q, k l2-normalized over the flattened spatial axis (n = 64*64 = 4096)
  sim  = 10 * q^T k   per (batch, head)      -> (n, n)
  attn = softmax(sim, axis=-1)
  out  = attn @ v^T
  y    = w_out @ out + b_out

Sharding: 16 (b, h) units over 8 cores -> each core owns batch c//2 and
heads {2*(c%2), 2*(c%2)+1}. The out-projection contracts over all heads,
so each core emits a partial y for its 64 hidden channels; the host sums
the two partials per batch and adds the bias.
"""

import sys

import numpy as np

for _p in ("/opt/trn_rl_repo",):
    if _p not in sys.path:
        sys.path.insert(0, _p)

import ml_dtypes  # noqa: E402

import concourse.mybir as mybir  # noqa: E402
from concourse import bacc  # noqa: E402
from concourse.bass_utils import run_bass_kernel_spmd  # noqa: E402
from concourse.tile import TileContext  # noqa: E402

BF16 = mybir.dt.bfloat16
F32 = mybir.dt.float32

B = 4          # batch
HEADS = 4      # global heads
DH = 32        # dim per head
DIM = 256      # channels
HID = HEADS * DH  # 128
N = 4096       # flattened spatial (64*64)
SCALE = 10.0

N_CORES = 8
LOCAL_HEADS = 2          # heads per core
IB = 512                 # i-block (query columns per pass)
N_IB = N // IB           # 8
JG = 512                 # j-group (4 j-blocks of 128)
N_JG = N // JG           # 8

LAST_RESULTS = None      # test harness reads exec_time_ns from here
ROW_TILING = True        # 4-way PE row tiling for sim matmuls


def _build_graph(reps=1):
    nc = bacc.Bacc(None, target_bir_lowering=False)

    x_d = nc.declare_dram_parameter("x", [DIM, N], BF16, isOutput=False)
    # [q_h0(128) | k_h0(128) | q_h1(128) | k_h1(128)] columns, rows = c
    wqk_d = nc.declare_dram_parameter("wqk", [DIM, 512], BF16, isOutput=False)
    # v weights transposed: rows = c, cols = [v_h0 d(32) | v_h1 d(32)]
    wv_d = nc.declare_dram_parameter("wv", [DIM, 2 * DH], BF16, isOutput=False)
    # out-proj weights transposed: rows = local hidden (64), cols = out c (256)
    wo_d = nc.declare_dram_parameter("wo", [2 * DH, DIM], BF16, isOutput=False)
    y_d = nc.declare_dram_parameter("y", [DIM, N], F32, isOutput=True)

    with TileContext(nc) as tc:
        with (
            tc.tile_pool(name="const", bufs=1) as cpool,
            tc.tile_pool(name="big", bufs=1) as bpool,
            tc.tile_pool(name="attn", bufs=8) as apool,
            tc.tile_pool(name="small", bufs=2) as spool,
            tc.tile_pool(name="ps_sim", bufs=1, space="PSUM") as ps_sim,
            tc.tile_pool(name="ps_out", bufs=3, space="PSUM") as ps_out,
        ):
            for _ in range(reps):
                _emit(nc, x_d, wqk_d, wv_d, wo_d, y_d,
                      cpool, bpool, apool, spool, ps_sim, ps_out, ps_out)
    nc.compile()
    return nc


def _emit(nc, x_d, wqk_d, wv_d, wo_d, y_d,
          cpool, bpool, apool, spool, ps_sim, ps_out, ps_bc):
    EXP = mybir.ActivationFunctionType.Exp
    COPY = mybir.ActivationFunctionType.Copy
    SQUARE = mybir.ActivationFunctionType.Square

    # ---- weights + x into SBUF -------------------------------------------
    wqk0 = cpool.tile([128, 512], BF16, tag="wqk0")
    wqk1 = cpool.tile([128, 512], BF16, tag="wqk1")
    nc.sync.dma_start(out=wqk0, in_=wqk_d[0:128, :])
    nc.sync.dma_start(out=wqk1, in_=wqk_d[128:256, :])
    wv0 = cpool.tile([128, 2 * DH], BF16, tag="wv0")
    wv1 = cpool.tile([128, 2 * DH], BF16, tag="wv1")
    nc.sync.dma_start(out=wv0, in_=wv_d[0:128, :])
    nc.sync.dma_start(out=wv1, in_=wv_d[128:256, :])
    wo = cpool.tile([2 * DH, DIM], BF16, tag="wo")
    nc.sync.dma_start(out=wo, in_=wo_d[:, :])
    ones = cpool.tile([1, DH], BF16, tag="ones")
    nc.vector.memset(ones, 1.0)

    xs = [cpool.tile([128, N], BF16, tag=f"x{cb}", name=f"x{cb}") for cb in range(2)]
    for cb in range(2):
        for q in range(8):
            eng = nc.sync if (q + cb) % 2 == 0 else nc.gpsimd
            eng.dma_start(
                out=xs[cb][:, q * 512:(q + 1) * 512],
                in_=x_d[cb * 128:(cb + 1) * 128, q * 512:(q + 1) * 512],
            )

    # ---- qkv projection ---------------------------------------------------
    # qk[h]: (128, 8192) bf16; partitions 32g:32g+32 hold [q_h | k_h] (d=32),
    # replicated over the 4 row-groups g for PE row-tiling.
    qk = [bpool.tile([128, 2 * N], BF16, tag=f"qk{h}", name=f"qk{h}") for h in range(2)]
    for h in range(2):
        for part in range(2):  # 0 = q, 1 = k
            mcol = 128 * (2 * h + part)
            for sl in range(N_IB):
                t = ps_out.tile([128, IB], F32, tag="o")
                nc.tensor.matmul(t, wqk0[:, mcol:mcol + 128],
                                 xs[0][:, sl * IB:(sl + 1) * IB],
                                 start=True, stop=False, tile_position=(0, 0))
                nc.tensor.matmul(t, wqk1[:, mcol:mcol + 128],
                                 xs[1][:, sl * IB:(sl + 1) * IB],
                                 start=False, stop=True, tile_position=(0, 0))
                dst = qk[h][:, part * N + sl * IB: part * N + (sl + 1) * IB]
                if h == 0 and sl % 2 == 1:
                    nc.scalar.activation(dst, t, COPY)
                else:
                    nc.vector.tensor_copy(dst, t)

    # ---- v^T with ones column --------------------------------------------
    # vt: (128, 66*32) bf16; per j-block jb cols [66jb..]: [v0 |1| v1 |1]
    # so each head h reads a contiguous, 0-based [v_h | ones] (33 cols).
    # Ones columns are set by two strided one-shot memsets; each j-block's
    # v0/v1 halves land in one 2D-AP copy that strides over the ones column.
    vt = bpool.tile([128, 66 * 32], BF16, tag="vt")
    vt3 = vt.rearrange("p (a b) -> p a b", b=66)
    nc.vector.memset(vt3[:, :, 32:33], 1.0)
    nc.vector.memset(vt3[:, :, 65:66], 1.0)
    for jb in range(32):
        t = ps_out.tile([128, 2 * DH], F32, tag="o")
        nc.tensor.matmul(t, xs[0][:, jb * 128:(jb + 1) * 128], wv0,
                         start=True, stop=False, tile_position=(0, 0))
        nc.tensor.matmul(t, xs[1][:, jb * 128:(jb + 1) * 128], wv1,
                         start=False, stop=True, tile_position=(0, 0))
        dst = vt[:, 66 * jb:66 * jb + 66].rearrange(
            "p (a b) -> p a b", a=2)[:, :, 0:32]
        src = t.rearrange("p (a b) -> p a b", a=2)
        if jb % 2 == 0:
            nc.vector.tensor_copy(dst, src)
        else:
            nc.scalar.activation(dst, src, COPY)

    # ---- l2 normalization over the spatial axis --------------------------
    # Both per-d-row factors rs_q[d] * rs_k[d] sit inside the d-contraction
    # of sim, so they merge onto the q side; k stays raw.
    sq = bpool.tile([128, N], BF16, tag="sq")
    INT = mybir.dt.int32
    for h in range(2):
        rss = []
        for part in range(2):
            i4 = 2 * h + part
            ss = spool.tile([128, 1], F32, tag=f"ss{i4}", name="ss")
            ssa = spool.tile([128, 1], F32, tag=f"ssa{i4}", name="ssa")
            nc.scalar.activation(sq[:, 0:N // 2],
                                 qk[h][:, part * N: part * N + N // 2],
                                 SQUARE, accum_out=ssa)
            nc.scalar.activation(sq[:, N // 2:],
                                 qk[h][:, part * N + N // 2:(part + 1) * N],
                                 SQUARE, accum_out=ss)
            nc.vector.tensor_add(ss, ss, ssa)
            # rsqrt on DVE: bit-trick seed + 2 Newton steps (~4e-6 rel,
            # far below bf16 storage precision)
            rs = spool.tile([128, 1], F32, tag=f"rs{i4}", name="rs")
            nc.vector.tensor_scalar(
                out=rs.bitcast(INT), in0=ss.bitcast(INT), scalar1=1,
                scalar2=None, op0=mybir.AluOpType.arith_shift_right)
            nc.vector.tensor_scalar(
                out=rs.bitcast(INT), in0=rs.bitcast(INT), scalar1=0,
                scalar2=None, op0=mybir.AluOpType.bitwise_not)
            nc.vector.tensor_scalar(
                out=rs.bitcast(INT), in0=rs.bitcast(INT),
                scalar1=0x5f3759df + 1, scalar2=None, op0=mybir.AluOpType.add)
            u = spool.tile([128, 1], F32, tag=f"u{i4}", name="u")
            w = spool.tile([128, 1], F32, tag=f"w{i4}", name="w")
            for _ in range(2):
                nc.vector.tensor_mul(u, rs, rs)
                nc.vector.tensor_mul(u, u, ss)
                nc.vector.tensor_scalar(
                    out=w, in0=u, scalar1=-0.5, scalar2=1.5,
                    op0=mybir.AluOpType.mult, op1=mybir.AluOpType.add)
                nc.vector.tensor_mul(rs, rs, w)
            rss.append(rs)
        rqk = spool.tile([128, 1], F32, tag=f"rqk{h}", name="rqk")
        nc.vector.tensor_mul(rqk, rss[0], rss[1])
        # first i-block's columns scaled first so ib0 sims start earlier
        nc.vector.tensor_scalar_mul(qk[h][:, 0:IB], qk[h][:, 0:IB], rqk)
        nc.vector.tensor_scalar_mul(qk[h][:, IB:N], qk[h][:, IB:N], rqk)

    # out_all: (64, N) bf16, rows 0-31 = head0 out d's, 32-63 = head1
    out_all = bpool.tile([64, N], BF16, tag="out_all")

    # ---- attention --------------------------------------------------------
    # Epilogues are staged and flushed inside the NEXT (h, ib) iteration so
    # PE never waits on DVE-produced operands: recip at sg0, broadcast +
    # divide at sg3 (recip long done), out-projection at sg8 (divide done).
    def make_epilogue(h, ib, out_ps, halves=1):
        state = {}
        hw_ = IB // halves

        def st0():
            recip = spool.tile([1, IB], F32, tag="recip", name="recip")
            recip_bf = spool.tile([1, IB], BF16, tag="recip_bf",
                                  name="recip_bf")
            for c in range(halves):
                cs = slice(c * hw_, (c + 1) * hw_)
                nc.vector.reciprocal(recip[:, cs], out_ps[32:33, cs])
                nc.vector.tensor_copy(recip_bf[:, cs], recip[:, cs])
            state["recip"] = recip_bf

        def st1():
            bc_ps = ps_bc.tile([128, IB], F32, tag="o", name="bc_ps")
            bc_sb = spool.tile([32, IB], F32, tag="bc_sb", name="bc_sb")
            for c in range(halves):
                cs = slice(c * hw_, (c + 1) * hw_)
                gsl = slice(ib * IB + c * hw_, ib * IB + (c + 1) * hw_)
                nc.tensor.matmul(bc_ps[0:32, cs], ones, state["recip"][:, cs],
                                 start=True, stop=True, tile_position=(0, 0))
                nc.vector.tensor_copy(bc_sb[:, cs], bc_ps[0:32, cs])
                nc.vector.tensor_mul(out_all[32 * h:32 * (h + 1), gsl],
                                     out_ps[0:32, cs], bc_sb[:, cs])

        def st2():
            if h == 1:  # out projection for this i-block
                for c in range(halves):
                    gsl = slice(ib * IB + c * hw_, ib * IB + (c + 1) * hw_)
                    for m in range(2):
                        yp = ps_out.tile([128, hw_], F32, tag="o", name="yp")
                        nc.tensor.matmul(yp, wo[:, m * 128:(m + 1) * 128],
                                         out_all[:, gsl], start=True,
                                         stop=True, tile_position=(0, 0))
                        ysb = spool.tile([128, hw_], F32, tag="ysb",
                                         name="ysb")
                        nc.vector.tensor_copy(ysb, yp)
                        nc.sync.dma_start(
                            out=y_d[m * 128:(m + 1) * 128, gsl], in_=ysb)
        return [st0, st1, st2]

    # Flat, software-pipelined group stream: emit exp(g) -> sims(g+1) ->
    # outs(g) so PE always has the next group's sims done before ACT needs
    # them, and ACT runs back-to-back exps.
    WIDTHS = [2] * 16  # sums to 32 j-blocks
    FLUSH_AT = {0: 0, 2: 1, 5: 2}  # local group index -> epilogue stage
    groups = []
    for ib in range(N_IB):
        for h in range(2):
            jb0 = 0
            for gi, width in enumerate(WIDTHS):
                groups.append((ib, h, gi, jb0, width))
                jb0 += width

    def emit_sims(g, slot):
        ib, h, gi, jb0, width = g
        isl = slice(ib * IB, (ib + 1) * IB)
        sim = ps_sim.tile([128, 2 * IB], F32, tag="sim", name="sim", bufs=2)
        for e in range(width):
            jb = jb0 + e
            gp = 32 * (jb % 4) if ROW_TILING else 0
            lh = qk[h][gp:gp + 32, N + jb * 128: N + (jb + 1) * 128]
            rh = qk[h][gp:gp + 32, isl]
            nc.tensor.matmul(sim[:, e * IB:(e + 1) * IB], lh, rh,
                             start=True, stop=True, tile_position=(gp, 0))
        return sim

    pending = []
    out_ps_cur = {}
    cur_sim = emit_sims(groups[0], 0)
    for n, g in enumerate(groups):
        ib, h, gi, jb0, width = g
        isl = slice(ib * IB, (ib + 1) * IB)
        at = apool.tile([128, 3 * IB], BF16, tag="at")
        nc.scalar.activation(at[:, 0:width * IB], cur_sim[:, 0:width * IB],
                             EXP, scale=SCALE)
        nxt_sim = (emit_sims(groups[n + 1], n + 1)
                   if n + 1 < len(groups) else None)
        if gi == 0:
            out_ps_cur[h] = ps_out.tile([33, IB], F32, tag="o", name="out_ps")
        out_ps = out_ps_cur[h]
        for e in range(width):
            jb = jb0 + e
            lh = vt[:, 66 * jb + 33 * h: 66 * jb + 33 * h + 33]
            nc.tensor.matmul(out_ps, lh, at[:, e * IB:(e + 1) * IB],
                             start=(jb == 0), stop=(jb == 31),
                             tile_position=(0, 0))
        if gi in FLUSH_AT:
            for st in pending:
                st[FLUSH_AT[gi]]()
            if FLUSH_AT[gi] == 2:
                pending.clear()
        if gi == len(WIDTHS) - 1:
            pending.append(make_epilogue(h, ib, out_ps))
        cur_sim = nxt_sim
    for stage in range(3):
        for st in pending:
            st[stage]()


def _prep_inputs(x, w_qkv, w_out):
    bf = ml_dtypes.bfloat16
    in_maps = []
    for c in range(N_CORES):
        b, p = c // 2, c % 2
        xb = np.ascontiguousarray(x[b].reshape(DIM, N)).astype(bf)
        cols = []
        for lh in range(LOCAL_HEADS):
            g = 2 * p + lh
            wq = w_qkv[32 * g:32 * (g + 1), :]            # (32, 256)
            wk = w_qkv[HID + 32 * g:HID + 32 * (g + 1), :]
            cols.append(np.tile(wq.T, (1, 4)))            # (256, 128)
            cols.append(np.tile(wk.T, (1, 4)))
        wqk = np.ascontiguousarray(np.concatenate(cols, axis=1)).astype(bf)
        wv = np.ascontiguousarray(
            w_qkv[2 * HID + 64 * p: 2 * HID + 64 * (p + 1), :].T).astype(bf)
        wo = np.ascontiguousarray(w_out[:, 64 * p:64 * (p + 1)].T).astype(bf)
        in_maps.append({"x": xb, "wqk": wqk, "wv": wv, "wo": wo})
    return in_maps


def kernel(x, w_qkv, w_out, b_out):
    global LAST_RESULTS
    x = np.asarray(x, dtype=np.float32)
    w_qkv = np.asarray(w_qkv, dtype=np.float32)
    w_out = np.asarray(w_out, dtype=np.float32)
    b_out = np.asarray(b_out, dtype=np.float32)

    nc = _build_graph()
    in_maps = _prep_inputs(x, w_qkv, w_out)
    res = run_bass_kernel_spmd(nc, in_maps, core_ids=list(range(N_CORES)))
    LAST_RESULTS = res

    y = np.empty((B, DIM, 64, 64), np.float32)
    for b in range(B):
        yb = res.results[2 * b]["y"] + res.results[2 * b + 1]["y"]
        y[b] = (yb + b_out[:, None]).reshape(DIM, 64, 64)
    return y


if __name__ == "__main__":
    rng = np.random.default_rng(0)
    x = rng.standard_normal((B, DIM, 64, 64), dtype=np.float32)
    w_qkv = rng.standard_normal((3 * HID, DIM), dtype=np.float32) / 16.0
    w_out = rng.standard_normal((DIM, HID), dtype=np.float32) / 12.0
    b_out = rng.standard_normal(DIM, dtype=np.float32) * 0.01
    y = kernel(x, w_qkv, w_out, b_out)
    print("ok", y.shape, y.dtype, float(np.abs(y).max()))

